# revision 26
# baseline (speedup 1.0000x reference)
"""DirectionalContrastiveLoss on 8 TRN2 NeuronCores (Bass/Tile), v2.

Data-parallel over the N=16384 anchor rows (2048 rows/core); the 4000-row
memory bank is replicated (padded to 4096 columns with zero features).

Device algorithm:
- PSUM holds SC*(sim - pos + B0) per [128-row tile x 4096 mem cols],
  SC = 184.664 = 2^7/ln2, B0 = 88.03 - schraudolph shift:
  * features as fp8e4m3 DoubleRow matmuls (K=256 in one pass, both sides
    scaled by sqrt(SC/TEMP) = 42.97),
  * label mask as bf16 -SC*1000*onehot(label) x onehot(mem_label)
    matmuls on per-chunk 32-row PE tile positions (4-way concurrent),
  * per-row bias SC*(B0 - pos) via a 22nd one-hot row (x valid-col
    indicator, so pad columns stay exactly 0).
- Split exp-sum: ACT exps cols [0,ACOLS) (scale=1/SC, bias=-B0) with
  accum_out; DVE turns cols [ACOLS,4096) into int16 = clamp(psum, 0,
  32512) whose bf16 bitcast IS Schraudolph exp (no scale needed - it's
  in the matmul); DVE+GpSimd reduce-sum the bitcast tile.
- Rows with sim-pos > ~89.5 saturate to huge-finite/inf -> the host's
  -log(1/(S+1+eps)+eps) clamps them to the reference's 18.42 value
  (dead rows), exactly matching the fp32 reference semantics.
Device exports per-row partial sums [128, 6*NT]; the host computes the
-log epilogue, masks, and the final scalar (exact fp64).
"""
from contextlib import ExitStack

import numpy as np
import ml_dtypes

TEMP = 0.1
POS_THRESH = 0.7
EPS = 1e-8
N, C, M, NLAB = 16384, 256, 4000, 21
MP = 4096                  # memory columns padded
NCORES = 8
RPC = N // NCORES          # 2048 rows per core
NT = RPC // 128            # 16 n-tiles per core
CHUNK = 256                # psum region granularity (DR matmul out cols)
NCH = MP // CHUNK          # 16 chunks per n-tile

SC = 128.0 / np.log(2.0)            # 184.6635
SHIFT = 0.0579                      # schraudolph centering (in ln2 units)
B0 = (127.0 - SHIFT) * np.log(2.0)  # 87.9896  (psum bias; ACT cancels it)
SQ = float(np.sqrt(SC / TEMP))      # 42.9725  (bf16 per-side scale)
CAPV = 32512.0                      # 0x7F00 -> bf16 1.66e38 (huge finite)
J = 512                             # psum chunk width (bank)

NU = 4                     # psum units of 1024 cols; unit 3 -> DVE path
UNIT = MP // NU            # 1024
BCOLS = UNIT               # cols converted by DVE (schraudolph)

_cache = {}


def _build():
    import concourse.bacc as bacc
    import concourse.tile as tile
    from concourse import mybir

    f32 = mybir.dt.float32
    bf16 = mybir.dt.bfloat16
    f8 = mybir.dt.float8e4
    i16 = mybir.dt.int16
    Alu = mybir.AluOpType
    Act = mybir.ActivationFunctionType
    X = mybir.AxisListType.X
    DR = mybir.MatmulPerfMode.DoubleRow

    nc = bacc.Bacc(None)

    # DRAM params. ext: [C, RPC] bf16 (2 K-tiles of 128 partitions).
    ext1_d = nc.declare_dram_parameter("ext1", [C, RPC], bf16, isOutput=False)
    ext2_d = nc.declare_dram_parameter("ext2", [C, RPC], bf16, isOutput=False)
    mem_d = nc.declare_dram_parameter("extmem", [C, MP], bf16, isOutput=False)
    eqa1_d = nc.declare_dram_parameter("eqanc1", [128, RPC], bf16, isOutput=False)
    eqa2_d = nc.declare_dram_parameter("eqanc2", [128, RPC], bf16, isOutput=False)
    eqm_d = nc.declare_dram_parameter("eqmem", [128, MP], bf16, isOutput=False)
    out_d = nc.declare_dram_parameter("out", [128, 8 * NT], f32, isOutput=True)

    T0C = 4  # tiles in the startup DMA chunk

    with tile.TileContext(nc) as tc, ExitStack() as ctx:
        consts = ctx.enter_context(tc.tile_pool(name="consts", bufs=1))
        psum = ctx.enter_context(tc.tile_pool(name="psum", bufs=1, space="PSUM"))
        sb = ctx.enter_context(tc.tile_pool(name="sb", bufs=3))

        # ---- resident inputs, ordered by first use ----
        # (branch 0, tile 0 needs: ext1 t<4, mem, eqa1 t<4, eqm)
        e1_k = [
            consts.tile([128, RPC], bf16, tag=f"e1_{i}", name=f"e1_{i}")
            for i in range(2)
        ]
        e2_k = [
            consts.tile([128, RPC], bf16, tag=f"e2_{i}", name=f"e2_{i}")
            for i in range(2)
        ]
        mem_k = [
            consts.tile([128, MP], bf16, tag=f"mem_{i}", name=f"mem_{i}")
            for i in range(2)
        ]
        eqa1 = consts.tile([128, RPC], bf16, tag="eqa1", name="eqa1")
        eqa2 = consts.tile([128, RPC], bf16, tag="eqa2", name="eqa2")
        eqm = consts.tile([128, MP], bf16, tag="eqm", name="eqm")

        # unit order is [3, 0, 1, 2]; load unit-3 memory columns first so
        # tile-0's first matmuls can start ~1MB sooner.
        t0c = T0C * 128
        u3 = slice(3 * UNIT, MP)
        nc.sync.dma_start(out=e1_k[0][:, 0:t0c], in_=ext1_d[0:128, 0:t0c])
        nc.sync.dma_start(out=mem_k[0][:, u3], in_=mem_d[0:128, u3])
        nc.sync.dma_start(out=eqa1[:, 0:t0c], in_=eqa1_d[:, 0:t0c])
        nc.sync.dma_start(out=eqm[:, u3], in_=eqm_d[:, u3])
        nc.sync.dma_start(out=e1_k[1][:, 0:t0c], in_=ext1_d[128:256, 0:t0c])
        nc.sync.dma_start(out=mem_k[1][:, u3], in_=mem_d[128:256, u3])
        for i in range(2):
            nc.sync.dma_start(
                out=mem_k[i][:, 0 : 3 * UNIT],
                in_=mem_d[128 * i : 128 * i + 128, 0 : 3 * UNIT],
            )
        nc.sync.dma_start(out=eqm[:, 0 : 3 * UNIT], in_=eqm_d[:, 0 : 3 * UNIT])
        for i in range(2):
            nc.sync.dma_start(
                out=e1_k[i][:, t0c:], in_=ext1_d[128 * i : 128 * i + 128, t0c:]
            )
        nc.sync.dma_start(out=eqa1[:, t0c:], in_=eqa1_d[:, t0c:])
        for i in range(2):
            nc.sync.dma_start(out=e2_k[i][:], in_=ext2_d[128 * i : 128 * i + 128, :])
        nc.sync.dma_start(out=eqa2[:], in_=eqa2_d[:])

        biasA = consts.tile([128, 1], f32, tag="biasA", name="biasA")
        nc.vector.memset(biasA[:], -B0)

        # per-unit row sums; SSD separate so ACT and DVE writes never
        # share a tile (avoids cross-engine dependency serialization).
        SSA = consts.tile([128, 3, 2, NT], f32, tag="SSA", name="SSA")
        SSD = consts.tile([128, 2, NT], f32, tag="SSD", name="SSD")

        UORD = [3, 0, 1, 2]  # unit 3 (DVE path) first

        # The reduce of tile t is emitted AFTER the convert of tile t+1:
        # the next tile's u3 matmuls wait on a coarse DVE progress counter,
        # so a reduce sitting between convert(t) and the wait target stalls
        # the PE by the reduce's duration.
        pending_red = [None]

        def flush_red():
            if pending_red[0] is not None:
                exb_p, sl = pending_red[0]
                nc.vector.reduce_sum(out=sl, in_=exb_p, axis=X)
                pending_red[0] = None

        for b, (ekt, eqa) in enumerate([(e1_k, eqa1), (e2_k, eqa2)]):
            for t in range(NT):
                tc0 = t * 128
                pu = {
                    u: psum.tile([128, UNIT], f32, tag=f"pu{u}", name=f"pu{u}_{b}_{t}")
                    for u in UORD
                }
                for kt in range(2):
                    lhsT = ekt[kt][:, tc0 : tc0 + 128]
                    for u in UORD:
                        for j in range(2):
                            o0 = j * J
                            nc.tensor.matmul(
                                pu[u][:, o0 : o0 + J],
                                lhsT,
                                mem_k[kt][:, u * UNIT + o0 : u * UNIT + o0 + J],
                                start=(kt == 0),
                                stop=False,
                            )
                # u3's masks first so the DVE convert unblocks earliest;
                # bands (tile positions) run 4-way concurrent.
                for u in UORD:
                    for j in range(2):
                        o0 = j * J
                        nc.tensor.matmul(
                            pu[u][:, o0 : o0 + J],
                            eqa[32 * u : 32 * u + NLAB + 1, tc0 : tc0 + 128],
                            eqm[
                                32 * u : 32 * u + NLAB + 1,
                                u * UNIT + o0 : u * UNIT + o0 + J,
                            ],
                            start=False,
                            stop=True,
                            tile_position=(32 * u, 0),
                        )

                # unit 3 -> DVE: int16 = clamp(psum, 0, 32512); bf16 bitcast
                # IS schraudolph exp; reduce the bitcast tile.
                ex = sb.tile([128, BCOLS], i16, tag="ex", name=f"ex{b}_{t}")
                nc.vector.tensor_scalar(
                    out=ex[:],
                    in0=pu[3][:],
                    scalar1=0.0,
                    scalar2=CAPV,
                    op0=Alu.max,
                    op1=Alu.min,
                )
                flush_red()
                pending_red[0] = (ex[:].bitcast(bf16), SSD[:, b, t : t + 1])

                # units 0-2 -> ACT: exp((psum/SC) - B0), accum row sum
                for u in (0, 1, 2):
                    nc.scalar.activation(
                        out=pu[u][:],
                        in_=pu[u][:],
                        func=Act.Exp,
                        bias=biasA[:],
                        scale=float(1.0 / SC),
                        accum_out=SSA[:, u, b, t : t + 1],
                    )

        flush_red()
        nc.sync.dma_start(
            out=out_d[:, 0 : 6 * NT], in_=SSA[:].rearrange("p u b t -> p (u b t)")
        )
        nc.sync.dma_start(
            out=out_d[:, 6 * NT : 8 * NT], in_=SSD[:].rearrange("p b t -> p (b t)")
        )

    nc.finalize()
    return nc


def _host_prep(inputs):
    bf = ml_dtypes.bfloat16
    f8 = ml_dtypes.float8_e4m3
    f1 = np.ascontiguousarray(np.asarray(inputs["output_feat1"], np.float32))
    f2 = np.ascontiguousarray(np.asarray(inputs["output_feat2"], np.float32))
    l1 = np.asarray(inputs["pseudo_label1"], np.int32)
    l2 = np.asarray(inputs["pseudo_label2"], np.int32)
    ul1 = np.asarray(inputs["output_ul1"], np.float32)
    ul2 = np.asarray(inputs["output_ul2"], np.float32)
    i1 = np.asarray(inputs["selected_idx1"], np.int64)
    i2 = np.asarray(inputs["selected_idx2"], np.int64)

    b, c, h, w = ul1.shape
    u1 = ul1.transpose(0, 2, 3, 1).reshape(b * h * w, c)
    u2 = ul2.transpose(0, 2, 3, 1).reshape(b * h * w, c)
    mem = np.concatenate([u1[i1], u2[i2]], axis=0)               # [M, C]
    memlab = np.concatenate([l1[i1], l2[i2]], axis=0)            # [M]

    pos = (f1 * f2).sum(axis=1, dtype=np.float64) / TEMP         # [N] exact

    extmem = np.zeros((C, MP), np.float32)
    extmem[:, :M] = mem.T * SQ
    extmem = extmem.astype(bf)                                   # [C, MP]

    # mask memory side: rows 32u+i = onehot(memlab==i); row 32u+21 = valid
    lab_eye = np.arange(NLAB, dtype=np.int32)
    eqmem = np.zeros((128, MP), np.float32)
    oh_mem = (memlab[None, :] == lab_eye[:, None]).astype(np.float32)
    for u in range(4):
        eqmem[32 * u : 32 * u + NLAB, :M] = oh_mem
        eqmem[32 * u + NLAB, :M] = 1.0
    eqmem = eqmem.astype(bf)

    def eq_anchor(lab, pos_sl):
        out = np.zeros((128, lab.shape[0]), np.float32)
        oh = (-SC * 1000.0) * (lab[None, :] == lab_eye[:, None])
        brow = SC * (B0 - pos_sl)
        for u in range(4):
            out[32 * u : 32 * u + NLAB] = oh
            out[32 * u + NLAB] = brow
        return out.astype(bf)

    def pack_ext(x):   # [RPC, C] fp32 -> [C, RPC] bf16
        return np.ascontiguousarray((x * SQ).T).astype(bf)

    in_maps = []
    for cix in range(NCORES):
        sl = slice(cix * RPC, (cix + 1) * RPC)
        in_maps.append({
            "ext1": pack_ext(f1[sl]),
            "ext2": pack_ext(f2[sl]),
            "extmem": extmem,
            "eqanc1": np.ascontiguousarray(eq_anchor(l1[sl], pos[sl])),
            "eqanc2": np.ascontiguousarray(eq_anchor(l2[sl], pos[sl])),
            "eqmem": eqmem,
        })
    return in_maps, pos


def _finalize(results, inputs):
    g1 = np.asarray(inputs["pseudo_logits1"], np.float64)
    g2 = np.asarray(inputs["pseudo_logits2"], np.float64)

    # device partials -> S per row, ordered [core, tile, lane]
    S = np.zeros((2, N), np.float64)
    for cix, r in enumerate(results):
        o = np.asarray(r["out"], np.float64)
        st = o[:, 0 : 6 * NT].reshape(128, 3, 2, NT).sum(axis=1)
        st += o[:, 6 * NT : 8 * NT].reshape(128, 2, NT)
        for b in range(2):
            # row (cix*RPC + t*128 + lane) <- st[lane, b, t]
            S[b, cix * RPC : (cix + 1) * RPC] = st[:, b].T.reshape(RPC)

    S = np.nan_to_num(S, nan=np.inf, posinf=np.inf, neginf=0.0)
    with np.errstate(divide="ignore", over="ignore"):
        sig = 1.0 / (S + 1.0 + EPS)
        lam = -np.log(sig + EPS)                     # per-row loss term

    m1 = ((g2 > POS_THRESH) & (g1 < g2)).astype(np.float64)
    m2 = ((g1 > POS_THRESH) & (g2 < g1)).astype(np.float64)
    loss = (lam[0] * m1).sum() / (m1.sum() + 1e-12) + \
           (lam[1] * m2).sum() / (m2.sum() + 1e-12)
    return np.float32(loss)


def _run(inputs, trace=False):
    from concourse.bass_utils import run_bass_kernel_spmd

    if "nc" not in _cache:
        _cache["nc"] = _build()
    in_maps, _pos = _host_prep(inputs)
    res = run_bass_kernel_spmd(
        _cache["nc"], in_maps, list(range(NCORES)), trace=trace
    )
    return _finalize(res.results, inputs), res


def kernel(**inputs):
    out, _ = _run(inputs)
    return out


def kernel_with_profile(**inputs):
    out, res = _run(inputs, trace=True)
    return out, res
